# revision 1
# baseline (speedup 1.0000x reference)
"""Trainium2 Bass kernel for nn_AdaptiveMixedCoding (8 NeuronCores).

Sharding: data-parallel over B_img (8 images per core); caps/cap_lens/alpha
replicated. Caption Grams are computed split across cores (8 captions each)
and AllGathered.

Per-core algorithm (Bi=8 imgs, R=36 regions, Bc=64 caps, W=50 words, D=1024):
  S[row, c, w] = dot(imgs[row], caps[c, w])   one [288,1024]x[1024,3200] bf16
                 matmul; a K=1 ones-row accumulates bc_addS (0 valid / -1e6
                 masked) into the same PSUM group
  G[c]         = caps_c caps_c^T   (pair block-diag layout: G_{2p} at
                 [0:50,0:50], G_{2p+1} at [50:100,50:100] of Gp[:, p, :])
  t            = (S + bc_addS) * bc_scale'    (bc_scale' = inv_nc valid /
                 0.01 masked -> masked t ~= -1e4, finite)
  softmax: rowmax / exp(scale=10*inv_ni) / den;  hard = (t - rowmax == 0)
  mixed'       = hard + soft * a/((1-a) den)           (= mixed/(1-a))
  num'         = sum_w mixed' * S_sb
  qf'          = mixed'^T G mixed'   (pair transposes -> M_T,
                 u = M_T^T Gp in row space, flat products, reduce)
  out          = num'/(sqrt(qf') + eps/(1-a)), invalid img rows -> -1

End-to-end l2 rel err vs the f32 reference ~1.4e-3 (bf16 matmul rounding).
"""
import sys
import contextlib

sys.path.insert(0, '/opt/trn_rl_repo')

import numpy as np
import ml_dtypes

from concourse import bacc, tile, mybir

F32 = mybir.dt.float32
BF16 = mybir.dt.bfloat16
AF = mybir.ActivationFunctionType
OP = mybir.AluOpType
AX = mybir.AxisListType

N_CORES = 8
B, R, W, D = 64, 36, 50, 1024
BC = B
BI = B // N_CORES
ROWS = BI * R               # 288
CW = BC * W                 # 3200
KC = D // 128               # 8 contraction chunks
NP = BC // 2                # 32 caption pairs
PPC = NP // N_CORES         # 4 pairs (8 captions) per core
CPC = BC // N_CORES         # 8 captions per core
GW = CPC * W                # 400 caption-word columns per core
ROW_TILES = [(0, 108), (108, 108), (216, 72)]
N_CHUNKS = [(i * 512, min(512, CW - i * 512)) for i in range((CW + 511) // 512)]
EPS = 1e-8
NEGS = -1e6                 # pre-scale mask offset; *0.01 -> -1e4 in t
KMASK = 0.01
TINY = 1e-30

_CACHE = {}


def _build(a: float):
    am = max(a, 1e-6)
    oma = max(1.0 - a, 1e-6)

    nc = bacc.Bacc("TRN2", target_bir_lowering=False, debug=False,
                   num_devices=N_CORES)

    capsT = nc.declare_dram_parameter("capsT", [D, CW], BF16, isOutput=False)
    gcaps = nc.declare_dram_parameter("gcaps", [D, GW], BF16, isOutput=False)
    imgsT = nc.declare_dram_parameter("imgsT", [D, ROWS], BF16, isOutput=False)
    imgs_nat = nc.declare_dram_parameter("imgs_nat", [ROWS, D], F32,
                                         isOutput=False)
    scale_row_in = nc.declare_dram_parameter("scale_row_in", [1, CW], F32,
                                             isOutput=False)  # invnc*mask+off
    adds_row = nc.declare_dram_parameter("adds_row", [1, CW], F32,
                                         isOutput=False)    # 0 / NEGS
    iv_col = nc.declare_dram_parameter("iv_col", [ROWS, 1], F32,
                                       isOutput=False)
    ivm1_col = nc.declare_dram_parameter("ivm1_col", [ROWS, 1], F32,
                                         isOutput=False)
    out_ext = nc.declare_dram_parameter("out", [BI, BC, R], F32, isOutput=True)
    import os
    DEBUG = bool(os.environ.get("KERNEL_DEBUG"))
    if DEBUG:
        dbg_bc = nc.declare_dram_parameter("dbg_bc", [128, CW], F32,
                                           isOutput=True)
        dbg_nsq = nc.declare_dram_parameter("dbg_nsq", [128, NP], F32,
                                            isOutput=True)

    gb_in = nc.dram_tensor("gb_in", [PPC, 100, 128], BF16)
    gb_out = nc.dram_tensor("gb_out", [NP, 100, 128], BF16,
                            addr_space="Shared")

    with tile.TileContext(nc) as tc, contextlib.ExitStack() as ctx:
        const = ctx.enter_context(tc.tile_pool(name="const", bufs=1))
        big = ctx.enter_context(tc.tile_pool(name="big", bufs=1))
        work = ctx.enter_context(tc.tile_pool(name="work", bufs=2))
        small = ctx.enter_context(tc.tile_pool(name="small", bufs=2))
        workm = ctx.enter_context(tc.tile_pool(name="workm", bufs=3))
        psS = ctx.enter_context(tc.tile_pool(name="psS", bufs=2, space="PSUM"))
        psM = ctx.enter_context(tc.tile_pool(name="psM", bufs=6, space="PSUM"))

        # ---- constants --------------------------------------------------
        ident_bf = const.tile([128, 128], BF16)
        from concourse.masks import make_identity
        make_identity(nc, ident_bf[:])
        ident_f32 = const.tile([128, 128], F32)
        make_identity(nc, ident_f32[:])
        ones_bf = const.tile([1, 128], BF16)
        nc.gpsimd.memset(ones_bf[:], 1.0)

        # ---- input loads ------------------------------------------------
        caps_sb = big.tile([128, KC, CW], BF16)
        for kc in range(KC):
            nc.sync.dma_start(out=caps_sb[:, kc, :],
                              in_=capsT[kc * 128:(kc + 1) * 128, :])
        gcaps_sb = big.tile([128, KC, GW], BF16)
        nc.sync.dma_start(out=gcaps_sb[:],
                          in_=gcaps.rearrange("(k p) m -> p k m", p=128))
        imgsT_sb = big.tile([128, KC, ROWS], BF16)
        nc.sync.dma_start(out=imgsT_sb[:],
                          in_=imgsT.rearrange("(k p) m -> p k m", p=128))

        addsrow_sb = const.tile([1, CW], BF16)
        nc.gpsimd.dma_start(out=addsrow_sb[:], in_=adds_row[:])

        # ---- Grams for this core's 8 captions, then AllGather -----------
        # Gloc: even cap at [0:50, j, 0:50], odd cap at [64:114, j, 50:100]
        Gloc = big.tile([128, PPC, 128], BF16)
        nc.vector.memset(Gloc[:], 0.0)
        for lc in range(CPC):
            mw = min(128, GW - lc * W)   # FWL pad when possible
            gps = psM.tile([128, W], F32, tag="ps")
            for kc in range(KC):
                nc.tensor.matmul(gps[:mw, :],
                                 gcaps_sb[:, kc, lc * W:lc * W + mw],
                                 gcaps_sb[:, kc, lc * W:(lc + 1) * W],
                                 start=(kc == 0), stop=(kc == KC - 1))
            j, dd = divmod(lc, 2)
            if dd == 0:
                nc.scalar.activation(Gloc[0:W, j, 0:W], gps[0:W, :], AF.Copy)
            else:
                nc.scalar.activation(Gloc[64:64 + W, j, 50:100], gps[0:W, :],
                                     AF.Copy)
        # scale_row comes precomputed from the host (tiny DMA, bf16 cast)
        scale_row = workm.tile([1, CW], BF16, tag="mixed")
        nc.gpsimd.dma_start(out=scale_row[:], in_=scale_row_in[:])

        # full Gram gather (overlaps the S matmuls; needed only by qf)
        zb = const.tile([128, PPC * 100], BF16)
        nc.vector.memset(zb[:], 0.0)
        nc.gpsimd.dma_start(
            out=gb_in.rearrange("j r b -> (j r b)")[None, :],
            in_=zb[:])
        nc.gpsimd.dma_start(
            out=gb_in[:, 0:50, 0:50].rearrange("j r b -> r j b"),
            in_=Gloc[0:50, :, 0:50])
        nc.gpsimd.dma_start(
            out=gb_in[:, 50:100, 50:100].rearrange("j r b -> r j b"),
            in_=Gloc[64:114, :, 50:100])
        nc.gpsimd.collective_compute(
            "AllGather", OP.bypass,
            replica_groups=[list(range(N_CORES))],
            ins=[gb_in[:].opt()],
            outs=[gb_out[:].opt()],
        )
        # Gp[:, p, :]: G_{2p} at [0:50, 0:50], G_{2p+1} at [50:100, 50:100]
        Gp = big.tile([128, NP, 128], BF16)
        nc.vector.memset(Gp[:], 0.0)
        for k in range(N_CORES):
            nc.sync.dma_start(
                out=Gp[0:100, k * PPC:(k + 1) * PPC, :],
                in_=gb_out[k * PPC:(k + 1) * PPC, :, :].rearrange(
                    "j r b -> r j b"))

        # transposed mixed, pair-block layout (built per row tile)
        M_T = big.tile([128, NP, ROWS], BF16)
        nc.vector.memset(M_T[:, NP - 1, :], 0.0)

        # broadcast to 128 partitions via ones-matmul
        bc_scale = big.tile([128, CW], F32)
        for (n0, nw) in N_CHUNKS:
            bps = psM.tile([128, 512], F32, tag="ps")
            nc.tensor.matmul(bps[:, :nw], ones_bf[:],
                             scale_row[:, n0:n0 + nw], start=True, stop=True)
            nc.scalar.activation(bc_scale[:, n0:n0 + nw], bps[:, :nw], AF.Copy)

        if DEBUG:
            nc.scalar.dma_start(out=dbg_bc[:], in_=bc_scale[:])
            nc.scalar.dma_start(out=dbg_nsq[:], in_=invnc[:])

        # ---- per row-tile pipeline --------------------------------------
        for (r0, rt) in ROW_TILES:
            fwl = (r0 + 128 <= ROWS)
            mm = 128 if fwl else rt     # matmul M (junk rows not evacuated)
            img_nat_t = work.tile([128, D], F32, tag="imgnat")
            nc.sync.dma_start(out=img_nat_t[:rt, :],
                              in_=imgs_nat[r0:r0 + rt, :])
            sq_scr = work.tile([128, D], F32, tag="t")
            nsq_img = small.tile([128, 1], F32, tag="nsqimg")
            nc.scalar.activation(sq_scr[:rt, :], img_nat_t[:rt, :], AF.Square,
                                 accum_out=nsq_img[:rt, :])
            invni10 = small.tile([128, 1], F32, tag="invni10")
            nc.scalar.activation(invni10[:rt, :], nsq_img[:rt, :], AF.Sqrt,
                                 scale=0.01)
            nc.vector.reciprocal(invni10[:rt, :], invni10[:rt, :])
            iv_t = small.tile([128, 1], F32, tag="ivt")
            nc.gpsimd.dma_start(out=iv_t[:rt, :], in_=iv_col[r0:r0 + rt, :])
            ivm1_t = small.tile([128, 1], F32, tag="ivm1t")
            nc.gpsimd.dma_start(out=ivm1_t[:rt, :],
                                in_=ivm1_col[r0:r0 + rt, :])

            # S matmul (+ bias row) -> psum; evac raw S and masked-scaled t
            t = work.tile([128, CW], F32, tag="t")
            S_sb = work.tile([128, CW], F32, tag="S_sb")
            for (n0, nw) in N_CHUNKS:
                sps = psS.tile([128, 512], F32, tag="sps")
                for kc in range(KC):
                    nc.tensor.matmul(sps[:mm, :nw],
                                     imgsT_sb[:, kc, r0:r0 + mm],
                                     caps_sb[:, kc, n0:n0 + nw],
                                     start=(kc == 0), stop=False)
                nc.tensor.matmul(sps[:mm, :nw], ones_bf[:, :mm],
                                 addsrow_sb[:, n0:n0 + nw],
                                 start=False, stop=True)
                nc.scalar.activation(S_sb[:rt, n0:n0 + nw], sps[:rt, :nw],
                                     AF.Copy)
                # read from SBUF so the PSUM bank frees after the evac alone
                # (keeps the PE running ahead during the bc_scale prologue)
                nc.vector.tensor_tensor(t[:rt, n0:n0 + nw],
                                        S_sb[:rt, n0:n0 + nw],
                                        bc_scale[:rt, n0:n0 + nw], OP.mult)

            t3 = t[:rt, :].rearrange("p (c w) -> p c w", w=W)
            rowmax = small.tile([128, BC], F32, tag="rowmax")
            nc.vector.tensor_reduce(rowmax[:rt, :], t3, axis=AX.X, op=OP.max)
            nc.vector.tensor_tensor(
                t3, t3, rowmax[:rt, :, None].to_broadcast([rt, BC, W]),
                OP.subtract)
            exp_l = workm.tile([128, CW], BF16, tag="expl")
            nc.scalar.activation(exp_l[:rt, :], t[:rt, :], AF.Exp,
                                 scale=invni10[:rt, :])
            el3 = exp_l[:rt, :].rearrange("p (c w) -> p c w", w=W)
            den = small.tile([128, BC], F32, tag="den")
            nc.vector.tensor_reduce(den[:rt, :], el3, axis=AX.X, op=OP.add)
            invden = small.tile([128, BC], F32, tag="invden")
            nc.vector.tensor_scalar(invden[:rt, :], den[:rt, :], oma / am,
                                    oma * TINY / am, OP.mult, OP.add)
            nc.vector.reciprocal(invden[:rt, :], invden[:rt, :])
            nc.vector.tensor_tensor(
                el3, el3, invden[:rt, :, None].to_broadcast([rt, BC, W]),
                OP.mult)
            mixed = workm.tile([128, CW], BF16, tag="mixed")
            nc.vector.scalar_tensor_tensor(mixed[:rt, :], t[:rt, :], 0.0,
                                           exp_l[:rt, :], OP.is_equal, OP.add)

            # num' = sum_w mixed * S  (bf16 product into exp_l)
            nc.vector.tensor_tensor(exp_l[:rt, :], mixed[:rt, :], S_sb[:rt, :],
                                    OP.mult)
            num = small.tile([128, BC], F32, tag="num")
            nc.vector.tensor_reduce(
                num[:rt, :], el3, axis=AX.X, op=OP.add)

            # qf': single 128-wide transposes per pair, u in row space,
            # flat products into exp_l, one reduce
            for p in range(NP):
                c0 = 100 * p
                tw = min(128, CW - c0)
                tps = psM.tile([128, 128], BF16, tag="ps")
                nc.tensor.transpose(tps[0:tw, :rt],
                                    mixed[:rt, c0:c0 + tw],
                                    ident_bf[0:rt, 0:rt])
                nc.scalar.activation(M_T[0:tw, p, r0:r0 + rt], tps[0:tw, :rt],
                                     AF.Copy)
            for p in range(NP):
                ups = psM.tile([128, 128], F32, tag="ps")
                nc.tensor.matmul(ups[:rt, :], M_T[:, p, r0:r0 + rt],
                                 Gp[:, p, :], start=True, stop=True)
                nc.vector.tensor_tensor(exp_l[:rt, 100 * p:100 * p + 100],
                                        mixed[:rt, 100 * p:100 * p + 100],
                                        ups[:rt, 0:100], OP.mult)
            qf = small.tile([128, BC], F32, tag="qf")
            nc.vector.tensor_reduce(
                qf[:rt, :], el3, axis=AX.X, op=OP.add)

            # out = num/(sqrt(qf) + eps'); invalid rows -> -1
            denom = small.tile([128, BC], F32, tag="denom")
            nc.scalar.activation(denom[:rt, :], qf[:rt, :], AF.Sqrt)
            nc.vector.tensor_scalar(denom[:rt, :], denom[:rt, :], EPS / oma,
                                    None, OP.add)
            nc.vector.reciprocal(denom[:rt, :], denom[:rt, :])
            res = small.tile([128, BC], F32, tag="res")
            nc.vector.tensor_tensor(res[:rt, :], num[:rt, :], denom[:rt, :],
                                    OP.mult)
            nc.vector.tensor_scalar(res[:rt, :], res[:rt, :], iv_t[:rt, :],
                                    ivm1_t[:rt, :], OP.mult, OP.add)

            ops_ = psM.tile([BC, 128], F32, tag="ps")
            nc.tensor.transpose(ops_[:, :rt], res[:rt, :],
                                ident_f32[0:rt, 0:rt])
            out_sb = work.tile([BC, 128], F32, tag="imgnat")
            nc.scalar.activation(out_sb[:, :rt], ops_[:, :rt], AF.Copy)
            ni = rt // R
            i0 = r0 // R
            nc.scalar.dma_start(
                out=out_ext[i0:i0 + ni, :, :].rearrange("i c r -> c i r"),
                in_=out_sb[:, :rt].rearrange("c (i r) -> c i r", r=R))

    nc.finalize()
    return nc


def _get_runner(a: float):
    key = round(float(a), 9)
    if key not in _CACHE:
        _CACHE[key] = _build(key)
    return _CACHE[key]


def _host_prep(imgs, caps, img_lens, cap_lens):
    imgs = np.ascontiguousarray(np.asarray(imgs, dtype=np.float32))
    caps = np.ascontiguousarray(np.asarray(caps, dtype=np.float32))
    img_lens = np.asarray(img_lens).astype(np.int64)
    cap_lens = np.asarray(cap_lens).astype(np.int64)

    capsT = np.ascontiguousarray(
        caps.reshape(BC * W, D).T).astype(ml_dtypes.bfloat16)   # [D, CW]
    cap_mask = (np.arange(W)[:, None] < cap_lens[None, :]).astype(np.float32)
    adds_row = np.where(cap_mask.T.reshape(1, CW) > 0, 0.0,
                        NEGS).astype(np.float32)
    # pair-block masks [w~ 128, pair 32]; blocks at rows [0:50] / [50:100]
    inv_nc = 1.0 / (np.linalg.norm(caps.astype(np.float64), axis=-1) + EPS)
    cm_cw = cap_mask.T.reshape(1, CW)
    scale_row_in = (inv_nc.reshape(1, CW) * cm_cw
                    + KMASK * (1.0 - cm_cw)).astype(np.float32)

    in_maps = []
    for core in range(N_CORES):
        sl = slice(core * BI, (core + 1) * BI)
        im = imgs[sl].reshape(ROWS, D)
        imT = np.ascontiguousarray(im.T).astype(ml_dtypes.bfloat16)
        iv = (np.arange(R)[None, :] < img_lens[sl][:, None]).astype(
            np.float32).reshape(ROWS, 1)
        in_maps.append({
            "capsT": capsT,
            "gcaps": np.ascontiguousarray(capsT[:, core * GW:(core + 1) * GW]),
            "imgsT": imT,
            "imgs_nat": im,
            "scale_row_in": scale_row_in,
            "adds_row": adds_row,
            "iv_col": iv,
            "ivm1_col": iv - 1.0,
        })
    return in_maps


def run_on_device(inputs: dict, trace: bool = False):
    """Returns (output [64,64,36] f32, BassKernelResults)."""
    from concourse.bass_utils import run_bass_kernel_spmd
    alpha = float(np.asarray(inputs["alpha"]).reshape(-1)[0])
    a = 1.0 / (1.0 + np.exp(-alpha))
    nc = _get_runner(a)
    in_maps = _host_prep(inputs["imgs"], inputs["caps"], inputs["img_lens"],
                         inputs["cap_lens"])
    r = run_bass_kernel_spmd(nc, in_maps, list(range(N_CORES)), trace=trace)
    out = np.concatenate([r.results[c]["out"][None] for c in range(N_CORES)],
                         axis=0)
    return out.reshape(B, BC, R).astype(np.float32), r


def kernel(imgs, caps, img_lens, cap_lens, alpha):
    out, _ = run_on_device({"imgs": imgs, "caps": caps, "img_lens": img_lens,
                            "cap_lens": cap_lens, "alpha": alpha})
    return out



# revision 9
# speedup vs baseline: 1.2458x; 1.2458x over previous
"""Trainium2 Bass kernel for nn_AdaptiveMixedCoding (8 NeuronCores).

Sharding: data-parallel over B_img (8 images per core); caps/cap_lens/alpha
replicated. Caption Grams computed split across cores (8 captions each),
AllGathered.

v2 design (per core: Bi=8 imgs, R=36 regions -> 288 rows; Bc=64 caps, W=50
words, D=1024):
  - caps are pre-NORMALIZED on host -> S matmul yields cosine*|img| directly;
    additive word mask (-6e4) folded in as a K=1 ones-row matmul.
  - S evacuated to SBUF as fp16 (10-bit mantissa keeps argmax ties rare);
    hard attention = is_equal(S, rowmax) with rowmax duplicated 2x so the
    compare runs in DVE 2x_1p mode.
  - soft part: exp on ScalarE (scale=10/|img| per row), den-reduce +
    num/qf reduces on the Pool engine (per-caption groups), mixed built as
    hard + (r/den)*exp in bf16 2x ops; then scaled by nc*mask (column
    broadcast materialized once via ones-matmul).
  - mixed written into a W=64-padded layout [128, 4096]; one XBAR DMA
    transpose gives all 32 caption-pair blocks [128(cw), 32, 128(rows)] for
    the Gram quadratic form; 32 PE matmuls vs pair-block-diagonal Gram,
    evac 4-up per PSUM bank; qf = per-caption reduce of mixed*u.
  - device outputs num and qf ([rows, 64] each); host does
    out = num/sqrt(qf) + invalid-row masking + layout transpose.
"""
import sys
import contextlib

sys.path.insert(0, '/opt/trn_rl_repo')

import numpy as np
import ml_dtypes

from concourse import bacc, tile, mybir

F32 = mybir.dt.float32
BF16 = mybir.dt.bfloat16
FP16 = mybir.dt.float16
AF = mybir.ActivationFunctionType
OP = mybir.AluOpType
AX = mybir.AxisListType

N_CORES = 8
B, R, W, D = 64, 36, 50, 1024
BC = B
BI = B // N_CORES
ROWS = BI * R               # 288
CW = BC * W                 # 3200
WP = 64                     # padded word slot
CWP = BC * WP               # 4096
KC = D // 128               # 8 contraction chunks
NP = BC // 2                # 32 caption pairs
CPC = BC // N_CORES         # 8 captions per core
GW = CPC * WP               # 512 gram columns per core (64-padded)
ROW_TILES = [(0, 108), (108, 108), (216, 72)]
N_CHUNKS = [(i * 512, min(512, CW - i * 512)) for i in range((CW + 511) // 512)]
EPS = 1e-8
NEGS = -60000.0             # additive word mask; fp16-safe

_CACHE = {}


def _build(r_mix: float):
    nc = bacc.Bacc("TRN2", target_bir_lowering=False, debug=False,
                   num_devices=N_CORES)

    capsT = nc.declare_dram_parameter("capsT", [D, CW], BF16, isOutput=False)
    gcaps = nc.declare_dram_parameter("gcaps", [D, GW], BF16, isOutput=False)
    imgsT = nc.declare_dram_parameter("imgsT", [D, ROWS], BF16, isOutput=False)
    adds_row = nc.declare_dram_parameter("adds_row", [1, CW], BF16,
                                         isOutput=False)    # 0 / NEGS
    ncm_row = nc.declare_dram_parameter("ncm_row", [1, CW], BF16,
                                        isOutput=False)     # nc_w * mask01
    invni_col = nc.declare_dram_parameter("invni_col", [ROWS, 1], F32,
                                          isOutput=False)   # 10/|img row|
    out_num = nc.declare_dram_parameter("out_num", [ROWS, BC], F32,
                                        isOutput=True)
    out_qf = nc.declare_dram_parameter("out_qf", [ROWS, BC], F32,
                                       isOutput=True)
    import os
    DEBUG = bool(os.environ.get("KERNEL_DEBUG"))
    if DEBUG:
        dbg = {}
        for nm, shape, dt in [
                ("dbg_S16", [128, CW], FP16), ("dbg_rmax", [128, BC], FP16),
                ("dbg_hard", [128, CW], BF16), ("dbg_exp", [128, CW], BF16),
                ("dbg_den", [128, BC], F32), ("dbg_mp", [128, CWP], BF16),
                ("dbg_MT", [128, NP, 128], BF16), ("dbg_U", [128, CWP], BF16),
                ("dbg_G", [128, NP, 128], BF16)]:
            dbg[nm] = nc.declare_dram_parameter(nm, shape, dt, isOutput=True)

    gb_in = nc.dram_tensor("gb_in", [NP // N_CORES, 128, 128], BF16)
    gb_out = nc.dram_tensor("gb_out", [NP, 128, 128], BF16,
                            addr_space="Shared")

    with tile.TileContext(nc) as tc, contextlib.ExitStack() as ctx:
        const = ctx.enter_context(tc.tile_pool(name="const", bufs=1))
        big = ctx.enter_context(tc.tile_pool(name="big", bufs=1))
        work = ctx.enter_context(tc.tile_pool(name="work", bufs=2))
        small = ctx.enter_context(tc.tile_pool(name="small", bufs=2))
        psS = ctx.enter_context(tc.tile_pool(name="psS", bufs=2, space="PSUM"))
        psU = ctx.enter_context(tc.tile_pool(name="psU", bufs=2, space="PSUM"))
        psG = ctx.enter_context(tc.tile_pool(name="psG", bufs=2, space="PSUM"))

        ones_bf = const.tile([1, 128], BF16)
        nc.gpsimd.memset(ones_bf[:], 1.0)

        # ---- input loads ------------------------------------------------
        imgsT_sb = big.tile([128, KC, ROWS], BF16)
        nc.sync.dma_start(out=imgsT_sb[:],
                          in_=imgsT.rearrange("(k p) m -> p k m", p=128))
        gcaps_sb = big.tile([128, KC, GW], BF16)
        nc.sync.dma_start(out=gcaps_sb[:],
                          in_=gcaps.rearrange("(k p) m -> p k m", p=128))
        addsrow_sb = const.tile([1, CW], BF16)
        nc.gpsimd.dma_start(out=addsrow_sb[:], in_=adds_row[:])
        ncmrow_sb = const.tile([1, CW], BF16)
        nc.gpsimd.dma_start(out=ncmrow_sb[:], in_=ncm_row[:])
        # caps: chunk-major so the first S matmul can start early
        caps_sb = big.tile([128, KC, CW], BF16)
        for (n0, nw) in N_CHUNKS:
            for kc in range(KC):
                nc.sync.dma_start(
                    out=caps_sb[:, kc, n0:n0 + nw],
                    in_=capsT[kc * 128:(kc + 1) * 128, n0:n0 + nw])

        # ---- caption Grams (normalized, 64-padded), AllGather -----------
        Gloc = big.tile([128, NP // N_CORES, 128], BF16)
        nc.vector.memset(Gloc[:], 0.0)
        for lc in range(CPC):
            gps = psG.tile([64, 64], F32, tag="g")
            for kc in range(KC):
                nc.tensor.matmul(gps[:],
                                 gcaps_sb[:, kc, lc * WP:(lc + 1) * WP],
                                 gcaps_sb[:, kc, lc * WP:(lc + 1) * WP],
                                 start=(kc == 0), stop=(kc == KC - 1))
            o = 64 * (lc & 1)
            nc.scalar.activation(Gloc[o:o + 64, lc >> 1, o:o + 64], gps[:],
                                 AF.Copy)
        nc.gpsimd.dma_start(out=gb_in.rearrange("j p m -> p j m"),
                            in_=Gloc[:])
        nc.gpsimd.collective_compute(
            "AllGather", OP.bypass,
            replica_groups=[list(range(N_CORES))],
            ins=[gb_in[:].opt()],
            outs=[gb_out[:].opt()],
        )
        Gp = big.tile([128, NP, 128], BF16)
        nc.sync.dma_start(out=Gp[:], in_=gb_out.rearrange("j p m -> p j m"))

        # ---- broadcast nc*mask row to 128 partitions --------------------
        ncmask = big.tile([128, CW], BF16)
        for (n0, nw) in N_CHUNKS:
            bps = psG.tile([128, 512], F32, tag="b")
            nc.tensor.matmul(bps[:, :nw], ones_bf[:],
                             ncmrow_sb[:, n0:n0 + nw], start=True, stop=True)
            nc.scalar.activation(ncmask[:, n0:n0 + nw], bps[:, :nw], AF.Copy)

        # mixed in padded layout; pads must be zero (memset once, the loop
        # only ever writes the [64,50] sub-views)
        mpad0 = big.tile([128, CWP], BF16, name="mpad0")
        mpad1 = big.tile([128, CWP], BF16, name="mpad1")
        mpad = [mpad0, mpad1]
        nc.vector.memset(mpad[0][:], 0.0)
        nc.vector.memset(mpad[1][:], 0.0)

        # ---- per row-tile pipeline --------------------------------------
        for ti, (r0, rt) in enumerate(ROW_TILES):
            mm = 128 if r0 + 128 <= ROWS else rt
            mp = mpad[ti % 2]
            invni_t = small.tile([128, 1], F32, tag="invni")
            nc.gpsimd.dma_start(out=invni_t[:rt, :],
                                in_=invni_col[r0:r0 + rt, :])

            # S matmul + additive mask -> fp16 S16
            S16 = work.tile([128, CW], FP16, tag="S16")
            for (n0, nw) in N_CHUNKS:
                sps = psS.tile([128, 512], F32, tag="s")
                for kc in range(KC):
                    nc.tensor.matmul(sps[:mm, :nw],
                                     imgsT_sb[:, kc, r0:r0 + mm],
                                     caps_sb[:, kc, n0:n0 + nw],
                                     start=(kc == 0), stop=False)
                nc.tensor.matmul(sps[:mm, :nw], ones_bf[:, :mm],
                                 addsrow_sb[:, n0:n0 + nw],
                                 start=False, stop=True)
                nc.scalar.activation(S16[:rt, n0:n0 + nw], sps[:rt, :nw],
                                     AF.Copy)

            if DEBUG and ti == 0:
                nc.scalar.dma_start(out=dbg["dbg_S16"][:], in_=S16[:])
            S3 = S16[:rt, :].rearrange("p (c w) -> p c w", w=W)
            # rowmax (fp16 out) + 2x duplication for the packed compare
            rmax = small.tile([128, BC], FP16, tag="rmax")
            nc.vector.tensor_reduce(rmax[:rt, :], S3, axis=AX.X, op=OP.max)
            rdup = small.tile([128, 2 * BC], FP16, tag="rdup")
            nc.vector.tensor_copy(
                rdup[:rt, :].rearrange("p (c t) -> p c t", t=2),
                rmax[:rt, :, None].to_broadcast([rt, BC, 2]))

            # hard = (S16 == rowmax)  [2x_1p: all fp16, dup-pair innermost]
            hard = work.tile([128, CW], BF16, tag="hard")
            nc.vector.tensor_tensor(
                hard[:rt, :].rearrange("p (c k t) -> p c k t", k=W // 2, t=2),
                S16[:rt, :].rearrange("p (c k t) -> p c k t", k=W // 2, t=2),
                rdup[:rt, :].rearrange("p (c t) -> p c t", t=2)[:, :, None, :]
                    .to_broadcast([rt, BC, W // 2, 2]),
                OP.is_equal)
            if DEBUG and ti == 0:
                nc.scalar.dma_start(out=dbg["dbg_hard"][:], in_=hard[:])

            # exp on ScalarE; den on Pool
            expv = work.tile([128, CWP], BF16, tag="expv")
            nc.scalar.activation(expv[:rt, 0:CW], S16[:rt, :], AF.Exp,
                                 scale=invni_t[:rt, :])
            if DEBUG and ti == 0:
                nc.scalar.dma_start(out=dbg["dbg_exp"][:], in_=expv[:, 0:CW])
            e3 = expv[:rt, 0:CW].rearrange("p (c w) -> p c w", w=W)
            den = small.tile([128, BC], F32, tag="den")
            nc.vector.tensor_reduce(den[:rt, :], e3, axis=AX.X, op=OP.add)
            invden = small.tile([128, BC], F32, tag="invden")
            nc.vector.reciprocal(invden[:rt, :], den[:rt, :])
            idup = small.tile([128, 2 * BC], BF16, tag="idup")
            nc.vector.tensor_scalar_mul(
                idup[:rt, :].rearrange("p (c t) -> p c t", t=2),
                invden[:rt, :, None].to_broadcast([rt, BC, 2]), r_mix)

            # mixed = hard + (r/den)*exp, then *= nc*mask -> padded layout
            nc.vector.tensor_tensor(
                expv[:rt, 0:CW].rearrange("p (c k t) -> p c k t", k=W // 2, t=2),
                expv[:rt, 0:CW].rearrange("p (c k t) -> p c k t", k=W // 2, t=2),
                idup[:rt, :].rearrange("p (c t) -> p c t", t=2)[:, :, None, :]
                    .to_broadcast([rt, BC, W // 2, 2]),
                OP.mult)
            nc.vector.tensor_tensor(expv[:rt, 0:CW], expv[:rt, 0:CW],
                                    hard[:rt, :], OP.add)
            m3 = mp[:rt, :].rearrange("p (c w) -> p c w", w=WP)[:, :, 0:W]
            nc.vector.tensor_tensor(m3, e3, ncmask[:rt, :].rearrange(
                "p (c w) -> p c w", w=W), OP.mult)

            # num = sum_w mixed * S  (product on DVE, reduce on Pool)
            nc.vector.tensor_tensor(
                hard[:rt, :].rearrange("p (c w) -> p c w", w=W),
                m3, S3, OP.mult)
            num = small.tile([128, BC], F32, tag="num")
            nc.vector.tensor_reduce(
                num[:rt, :],
                hard[:rt, :].rearrange("p (c w) -> p c w", w=W),
                axis=AX.X, op=OP.add)

            # all 32 pair-block transposes in one XBAR DMA
            M_T = work.tile([128, NP, 128], BF16, tag="MT")
            nc.sync.dma_start_transpose(out=M_T[:], in_=mp[:])

            # u = G * mixed per pair; 4 pairs per PSUM bank, evac to U
            U = work.tile([128, CWP], BF16, tag="U")
            for g in range(NP // 4):
                ups = psU.tile([128, 512], F32, tag="u")
                for jj in range(4):
                    j = 4 * g + jj
                    nc.tensor.matmul(ups[:, 128 * jj:128 * (jj + 1)],
                                     M_T[:, j, :], Gp[:, j, :],
                                     start=True, stop=True)
                nc.scalar.activation(U[:rt, 512 * g:512 * (g + 1)],
                                     ups[:rt, :], AF.Copy)

            # qf = sum_w mixed * u  (full-width product, Pool group reduce)
            nc.vector.tensor_tensor(expv[:rt, :], mp[:rt, :], U[:rt, :],
                                    OP.mult)
            qf = small.tile([128, BC], F32, tag="qf")
            nc.vector.tensor_reduce(
                qf[:rt, :],
                expv[:rt, :].rearrange("p (c w) -> p c w", w=WP)[:, :, 0:W],
                axis=AX.X, op=OP.add)

            nc.scalar.dma_start(out=out_num[r0:r0 + rt, :], in_=num[:rt, :])
            nc.scalar.dma_start(out=out_qf[r0:r0 + rt, :], in_=qf[:rt, :])
            if DEBUG and ti == 0:
                nc.scalar.dma_start(out=dbg["dbg_rmax"][:], in_=rmax[:])
                nc.scalar.dma_start(out=dbg["dbg_den"][:], in_=den[:])
                nc.scalar.dma_start(out=dbg["dbg_mp"][:], in_=mp[:])
                nc.scalar.dma_start(out=dbg["dbg_MT"][:], in_=M_T[:])
                nc.scalar.dma_start(out=dbg["dbg_U"][:], in_=U[:])
                nc.scalar.dma_start(out=dbg["dbg_G"][:], in_=Gp[:])

    nc.finalize()
    return nc


def _get_runner(r_mix: float):
    key = round(float(r_mix), 9)
    if key not in _CACHE:
        _CACHE[key] = _build(key)
    return _CACHE[key]


def _host_prep(imgs, caps, img_lens, cap_lens):
    imgs = np.ascontiguousarray(np.asarray(imgs, dtype=np.float32))
    caps = np.ascontiguousarray(np.asarray(caps, dtype=np.float32))
    cap_lens = np.asarray(cap_lens).astype(np.int64)

    ncn = np.linalg.norm(caps, axis=-1) + EPS            # [Bc, W]
    cn = caps / ncn[..., None]
    capsT = np.ascontiguousarray(
        cn.reshape(CW, D).T).astype(ml_dtypes.bfloat16)  # [D, CW]
    cap_mask = (np.arange(W)[None, :] < cap_lens[:, None])  # [Bc, W]
    adds_row = np.where(cap_mask.reshape(1, CW), 0.0,
                        NEGS).astype(ml_dtypes.bfloat16)
    ncm_row = np.where(cap_mask, ncn, 0.0).reshape(1, CW).astype(
        ml_dtypes.bfloat16)
    inv_ni = (10.0 / (np.linalg.norm(imgs, axis=-1) + EPS)).astype(
        np.float32)                                      # [B, R]

    in_maps = []
    for core in range(N_CORES):
        sl = slice(core * BI, (core + 1) * BI)
        imT = np.ascontiguousarray(
            imgs[sl].reshape(ROWS, D).T).astype(ml_dtypes.bfloat16)
        gc = np.zeros((CPC, WP, D), np.float32)
        gc[:, :W, :] = cn[core * CPC:(core + 1) * CPC]
        gcT = np.ascontiguousarray(
            gc.reshape(GW, D).T).astype(ml_dtypes.bfloat16)
        in_maps.append({
            "capsT": capsT,
            "gcaps": gcT,
            "imgsT": imT,
            "adds_row": adds_row,
            "ncm_row": ncm_row,
            "invni_col": np.ascontiguousarray(
                inv_ni[sl].reshape(ROWS, 1)),
        })
    return in_maps


def run_on_device(inputs: dict, trace: bool = False):
    """Returns (output [64,64,36] f32, BassKernelResults)."""
    from concourse.bass_utils import run_bass_kernel_spmd
    alpha = float(np.asarray(inputs["alpha"]).reshape(-1)[0])
    a = 1.0 / (1.0 + np.exp(-alpha))
    r_mix = a / max(1.0 - a, 1e-9)
    nc = _get_runner(r_mix)
    in_maps = _host_prep(inputs["imgs"], inputs["caps"], inputs["img_lens"],
                         inputs["cap_lens"])
    r = run_bass_kernel_spmd(nc, in_maps, list(range(N_CORES)), trace=trace)
    img_lens = np.asarray(inputs["img_lens"]).astype(np.int64)
    iv = (np.arange(R)[None, :] < img_lens[:, None])     # [B, R]
    outs = []
    for c in range(N_CORES):
        num = r.results[c]["out_num"].astype(np.float32)  # [ROWS, BC]
        qf = r.results[c]["out_qf"].astype(np.float32)
        o = num / (np.sqrt(np.maximum(qf, 0.0)) + 1e-30)
        o = o.reshape(BI, R, BC).transpose(0, 2, 1)       # [BI, BC, R]
        o = np.where(iv[c * BI:(c + 1) * BI, None, :], o, -1.0)
        outs.append(o)
    return np.concatenate(outs, axis=0).astype(np.float32), r


def kernel(imgs, caps, img_lens, cap_lens, alpha):
    out, _ = run_on_device({"imgs": imgs, "caps": caps, "img_lens": img_lens,
                            "cap_lens": cap_lens, "alpha": alpha})
    return out
